# revision 22
# baseline (speedup 1.0000x reference)
"""Trainium2 Bass kernel for DoubleBinaryLinear:
    y = ((x * s0) @ B.T * s2) @ A.T * s4 + bias
with x [4, 2048, 4096] fp32 and binary (+-1) B, A [4096, 4096].

v6a: fused-weight restructure, output-sharded across the 8 cores.

    M.T = diag(s0) B.T (diag(s2) A.T)   # [in, out], token-independent
    y   = x @ M.T * s4 + bias           # x pre-cast fp16 on host

Core c computes M.T columns for its 512 output rows (one 4096x4096x512
matmul, ~220 us) and keeps M.T resident in SBUF, then streams ALL 8192
tokens through a single fused matmul (~440 us). Out-sharding makes each
core's M shard exactly what its own x-pass needs -- no collective, and
the token loop has no per-tile activations (s0 rides the M eviction
scale). All matmuls single-pass fp16 (binary weights exact; M and x
rounded once, rel err ~4e-4 << 2e-2 gate).

v6a over v5: a2 = s2*A.T is precomputed on host and shipped fp16 (kills
the on-device phase A: 32 fp8 loads + scales and their latency chain),
and the final token chunk is split in two so the tail eviction+store is
half as long.

Per-matmul moving operands must come from plain tiles: slicing a wider
(batched-DMA) tile gives the operand AP a partition stride larger than
its free extent, which drops TensorE off its fast path (measured 259 ns
vs 216 ns per 512-row matmul). So x DMAs are one trigger per tile;
evictions run on the Vector engine.
"""

import os

import numpy as np
import ml_dtypes

import concourse.bacc as bacc
import concourse.mybir as mybir
from concourse import tile
from concourse import bass_utils

P = 128
F32 = mybir.dt.float32
F16 = mybir.dt.float16
F8 = mybir.dt.float8e4

IN_D = 4096
MID_D = 4096
OUT_D = 4096
BATCH = 4
SEQ = 2048
N_CORES = 8
T_ALL = BATCH * SEQ                 # 8192 tokens, every core sees all
OS = OUT_D // N_CORES               # 512 output rows per core
TC = 512                            # matmul moving free dim
nI = IN_D // P                      # 32 in tiles
nM = MID_D // P                     # 32 mid tiles
nOB = OS // P                       # 4 out blocks per core
nTC = T_ALL // TC                   # 16 token chunks
IG = 4                              # in-tiles per M-compute PSUM group

mult = mybir.AluOpType.mult
add = mybir.AluOpType.add


def _build_nc():
    nc = bacc.Bacc(None, target_bir_lowering=False)
    xTd = nc.dram_tensor("xT", [IN_D, T_ALL], F16, kind="ExternalInput")
    Bd = nc.dram_tensor("B", [MID_D, IN_D], F8, kind="ExternalInput")
    a2d = nc.dram_tensor("a2", [MID_D, OS], F16, kind="ExternalInput")
    nSC = nI + 2 * nOB
    scd = nc.dram_tensor("sc", [P, nSC], F32, kind="ExternalInput")
    yTd = nc.dram_tensor("yT", [OS, T_ALL], F32, kind="ExternalOutput")

    with tile.TileContext(nc) as tc:
        with (
            tc.tile_pool(name="consts", bufs=1) as cpool,
            tc.tile_pool(name="a2buf", bufs=1) as apool,
            tc.tile_pool(name="mtbuf", bufs=1) as mpool,
            tc.tile_pool(name="xin", bufs=2) as xpool,
            tc.tile_pool(name="bwts", bufs=44) as bpool,
            tc.tile_pool(name="yout", bufs=6) as ypool,
            tc.tile_pool(name="psum", bufs=8, space="PSUM") as pspool,
        ):
            # a2 = fp16(s2 * A.T) tiles [128 mid, OS], precomputed on host.
            # These feed the very first matmuls, so they go first on the
            # scalar queue; sc is only needed at the first mt eviction.
            a2 = []
            for mk in range(nM):
                a2t = apool.tile([P, OS], F16, tag=f"a{mk}", name=f"a{mk}")
                nc.scalar.dma_start(a2t[:], a2d[mk * P:(mk + 1) * P, :])
                a2.append(a2t)

            sc_t = cpool.tile([P, nSC], F32, tag="sc")
            nc.scalar.dma_start(sc_t[:], scd[:, :])
            s0_t = sc_t[:, 0:nI]
            s4_t = sc_t[:, nI:nI + nOB]
            bi_t = sc_t[:, nI + nOB:nSC]

            # Chunk-0 x tiles are only needed from ~40us (first interleave
            # slot fires in phase-B group 1); gate their DMA stream on a2[20]
            # so their HBM traffic stays out of the contended first ~22us
            # (a2 + B streams already demand ~320 GB/s of the 358 GB/s
            # per-core budget there; a2[31] gating measured slower).
            gate = cpool.tile([P, 1], F16, tag="gate")
            nc.gpsimd.tensor_copy(gate[:], a2[20][:, 0:1])
            x0 = []
            for it in range(nI):
                xt = xpool.tile([P, TC], F16, tag=f"x{it}", name=f"x{it}")
                nc.gpsimd.dma_start(xt[:], xTd[it * P:(it + 1) * P, 0:TC])
                x0.append(xt)
            # Chunk-0 accumulators held across all of phase B (4 banks);
            # phase B itself rings through the other 4.
            ps0 = [pspool.tile([P, TC], F32, tag="ps0", name="ps0", bufs=4)
                   for _ in range(nOB)]

            def x0_mm(j, ig):
                # j-th (0..15) interleave slot while phase-B group ig runs:
                # chunk-0 matmul for an in-tile of group ig-1.
                it = (ig - 1) * IG + j // nOB
                ob = j % nOB
                nc.tensor.matmul(ps0[ob][:], mt[it][:, ob * P:(ob + 1) * P],
                                 x0[it][:], start=(it == 0),
                                 stop=(it == nI - 1))

            # phase B: M.T tiles [128 in, OS]; s0 folded into eviction.
            # One chunk-0 matmul per two B-steps stretches the B-stream
            # demand timeline ~11% so DMA jitter stops stalling TensorE.
            mt = [mpool.tile([P, OS], F16, tag=f"m{it}", name=f"m{it}")
                  for it in range(nI)]
            for ig in range(nI // IG):
                psb = [pspool.tile([P, OS], F32, tag="ps", name="ps", bufs=4)
                       for _ in range(IG)]
                for mk in range(nM):
                    bt = bpool.tile([P, IG * P], F8, tag="wb")
                    nc.sync.dma_start(
                        bt[:], Bd[mk * P:(mk + 1) * P,
                                  ig * IG * P:(ig + 1) * IG * P])
                    last = mk == nM - 1
                    for t_ in range(IG):
                        nc.tensor.matmul(psb[t_][:], bt[:, t_ * P:(t_ + 1) * P],
                                         a2[mk][:], start=(mk == 0), stop=last)
                    if ig > 0 and mk % 2 == 1:
                        x0_mm(mk // 2, ig)
                for t_ in range(IG):
                    it = ig * IG + t_
                    nc.vector.tensor_scalar_mul(mt[it][:], psb[t_][:],
                                                s0_t[:, it:it + 1])
            for j in range(16):
                x0_mm(j, nI // IG)
            for ob in range(nOB):
                yt = ypool.tile([P, TC], F32, tag="yt")
                nc.vector.tensor_scalar(
                    yt[:], ps0[ob][:], s4_t[:, ob:ob + 1], bi_t[:, ob:ob + 1],
                    mult, add)
                nc.sync.dma_start(yTd[ob * P:(ob + 1) * P, 0:TC], yt[:])

            # phase C: stream remaining tokens; alternate the two 4-bank
            # PSUM rings so consecutive chunks never wait on evictions.
            # The final chunk is split in two 256-token halves so the tail
            # (eviction + store after the last matmul) is half as long.
            chunks = [(c * TC, TC) for c in range(1, nTC - 1)]
            chunks += [(15 * TC, TC // 2), (15 * TC + TC // 2, TC // 2)]
            for ci, (t0, w) in enumerate(chunks):
                xts = []
                for it in range(nI):
                    xt = xpool.tile([P, w], F16, tag=f"x{it}", name=f"x{it}")
                    nc.scalar.dma_start(
                        xt[:], xTd[it * P:(it + 1) * P, t0:t0 + w])
                    xts.append(xt)
                tag = "ps0" if ci % 2 else "ps"
                pso = [pspool.tile([P, TC], F32, tag=tag, name="pso", bufs=4)
                       for _ in range(nOB)]
                for it in range(nI):
                    for ob in range(nOB):
                        nc.tensor.matmul(pso[ob][:, 0:w],
                                         mt[it][:, ob * P:(ob + 1) * P],
                                         xts[it][:], start=(it == 0),
                                         stop=(it == nI - 1))
                for ob in range(nOB):
                    yt = ypool.tile([P, TC], F32, tag="yt")
                    nc.vector.tensor_scalar(
                        yt[:, 0:w], pso[ob][:, 0:w], s4_t[:, ob:ob + 1],
                        bi_t[:, ob:ob + 1], mult, add)
                    nc.sync.dma_start(
                        yTd[ob * P:(ob + 1) * P, t0:t0 + w], yt[:, 0:w])

    nc.compile()
    return nc


_NC_CACHE = None


def _get_nc():
    global _NC_CACHE
    if _NC_CACHE is None:
        _NC_CACHE = _build_nc()
    return _NC_CACHE


def _col_major(v):
    return np.ascontiguousarray(
        np.asarray(v, dtype=np.float32).reshape(-1, P).T)


def make_in_maps(x, scaling0, B, scaling2, A, scaling4, bias):
    xh = np.asarray(x, dtype=np.float32).reshape(T_ALL, IN_D).astype(np.float16)
    xT = np.ascontiguousarray(xh.T)
    B8 = np.asarray(B, dtype=np.float32).astype(ml_dtypes.float8_e4m3)
    a2_full = (np.asarray(scaling2, dtype=np.float32)[:, None]
               * np.asarray(A, dtype=np.float32).T).astype(np.float16)
    s0c = _col_major(scaling0)

    in_maps = []
    for c in range(N_CORES):
        sh = slice(c * OS, (c + 1) * OS)
        sc = np.ascontiguousarray(np.concatenate(
            [s0c, _col_major(np.asarray(scaling4)[sh]),
             _col_major(np.asarray(bias)[sh])], axis=1))
        in_maps.append({
            "xT": xT, "B": B8,
            "a2": np.ascontiguousarray(a2_full[:, sh]),
            "sc": sc,
        })
    return in_maps


def _unshard(results):
    y = np.empty((T_ALL, OUT_D), dtype=np.float32)
    for c in range(N_CORES):
        y[:, c * OS:(c + 1) * OS] = results[c]["yT"].T
    return y.reshape(BATCH, SEQ, OUT_D)


def kernel(x, scaling0, B, scaling2, A, scaling4, bias):
    # The profile hook isn't available in every environment; force the
    # plain execution path.
    os.environ.setdefault("BASS_NEVER_TRACE", "1")

    in_maps = make_in_maps(x, scaling0, B, scaling2, A, scaling4, bias)
    nc = _get_nc()
    res = bass_utils.run_bass_kernel_spmd(
        nc, in_maps, core_ids=list(range(N_CORES)))
    return _unshard(res.results)
